# revision 5
# baseline (speedup 1.0000x reference)
"""Trainium2 Bass kernel for nn_CoarseEdgeCoarsenAndFuse.

Full inputs in, full outputs out. Internally: 8-way shard over
(batch b in {0,1}) x (m-block of 96 out of N=384). Each core:
  - semantic einsum  Z_sem[z, a] = sum_n Zm[n, m, z] * A3[n, a]   (PE, fp16)
  - geometry RBF     rbf[k, m, a] = exp(-((d[m,a]-c_k)/w)^2 / 2)  (ACT)
  - fuse MLP         Linear -> LayerNorm -> SiLU -> Linear        (PE/ACT/DVE)
LayerNorm stats are computed with PE matmuls against constant
(I - J/128) and (J/128) matrices so every engine stays 128 wide.
Outputs are written fp16 in transposed layouts and assembled on host.
"""

import sys

sys.path.insert(0, "/opt/trn_rl_repo")

import time
from contextlib import ExitStack

import numpy as np

import concourse.bass as bass
import concourse.mybir as mybir
from concourse.bass_utils import run_bass_kernel_spmd
from concourse.tile import TileContext

F16 = mybir.dt.float16
F32 = mybir.dt.float32
AF = mybir.ActivationFunctionType

B, N, K, CZ, COUT, BINS = 2, 384, 384, 128, 128, 16
DMAX = 10.0
EPS = 1e-8
LN_EPS = 1e-5
MB = 96          # m-block per core
NB = 3           # n blocks of 128
G = 12           # m groups of 8 per core
WIDTH = DMAX / (BINS - 1)

LAST_RUN_SECONDS = None


def _build_program(apply_mask_c: bool):
    nc = bass.Bass("TRN2", target_bir_lowering=False, debug=False, num_devices=8)

    a3_e = nc.declare_dram_parameter("a3", [NB, 128, K], F16, isOutput=False)
    z_e = nc.declare_dram_parameter("z", [N, MB * CZ], F16, isOutput=False)
    dmat_e = nc.declare_dram_parameter("dmat", [MB, K], F32, isOutput=False)
    cvec_e = nc.declare_dram_parameter("cvec", [128, 8], F32, isOutput=False)
    w1a_e = nc.declare_dram_parameter("w1a", [CZ, COUT], F16, isOutput=False)
    w1b_e = nc.declare_dram_parameter("w1b", [BINS, COUT], F16, isOutput=False)
    w2_e = nc.declare_dram_parameter("w2", [COUT, COUT], F16, isOutput=False)
    cmat_e = nc.declare_dram_parameter("cmat", [128, 128], F16, isOutput=False)
    jmat_e = nc.declare_dram_parameter("jmat", [128, 128], F16, isOutput=False)
    if apply_mask_c:
        mcm_e = nc.declare_dram_parameter("mcm", [128, G], F32, isOutput=False)
        mca_e = nc.declare_dram_parameter("mca", [128, K], F32, isOutput=False)

    zsem_o = nc.declare_dram_parameter("zsem_o", [MB, CZ, K], F16, isOutput=True)
    zc_o = nc.declare_dram_parameter("zc_o", [MB, COUT, K], F16, isOutput=True)
    geo_o = nc.declare_dram_parameter("geo_o", [BINS, MB, K], F16, isOutput=True)

    with TileContext(nc) as tc:
        with ExitStack() as ctx:
            const = ctx.enter_context(tc.tile_pool(name="const", bufs=1))
            zpool = ctx.enter_context(tc.tile_pool(name="zpool", bufs=3))
            work = ctx.enter_context(tc.tile_pool(name="work", bufs=3))
            ps_a = ctx.enter_context(tc.tile_pool(name="ps_a", bufs=2, space="PSUM"))
            ps_b = ctx.enter_context(tc.tile_pool(name="ps_b", bufs=1, space="PSUM"))

            # ---- resident constants ----
            a3_t = []
            for nb in range(NB):
                t = const.tile([128, K], F16, tag=f"a3_{nb}")
                nc.sync.dma_start(out=t, in_=a3_e[nb])
                a3_t.append(t)
            w1a_t = const.tile([CZ, COUT], F16, tag="w1a")
            nc.sync.dma_start(out=w1a_t, in_=w1a_e[:])
            w1b_t = const.tile([BINS, COUT], F16, tag="w1b")
            nc.sync.dma_start(out=w1b_t, in_=w1b_e[:])
            w2_t = const.tile([COUT, COUT], F16, tag="w2")
            nc.sync.dma_start(out=w2_t, in_=w2_e[:])
            cmat_t = const.tile([128, 128], F16, tag="cmat")
            nc.sync.dma_start(out=cmat_t, in_=cmat_e[:])
            jmat_t = const.tile([128, 128], F16, tag="jmat")
            nc.sync.dma_start(out=jmat_t, in_=jmat_e[:])
            cvec_t = const.tile([128, 8], F32, tag="cvec")
            nc.sync.dma_start(out=cvec_t, in_=cvec_e[:])
            geo_bias = cvec_t[:, 0:1]
            b1c = cvec_t[:, 1:2]
            b2c = cvec_t[:, 2:3]
            gc = cvec_t[:, 3:4]
            bc = cvec_t[:, 4:5]
            epsc = cvec_t[:, 5:6]
            if apply_mask_c:
                mcm_t = const.tile([128, G], F32, tag="mcm")
                nc.sync.dma_start(out=mcm_t, in_=mcm_e[:])
                mca_t = const.tile([128, K], F32, tag="mca")
                nc.sync.dma_start(out=mca_t, in_=mca_e[:])

            # ---- geometry phase ----
            # rbf_fast partition p = k*8 + mi, free col = g*384 + a
            rbf_fast = const.tile([128, G * K], F16, tag="rbf_fast")
            for g in range(G):
                src = dmat_e[8 * g : 8 * g + 8, :]
                bsrc = bass.AP(
                    tensor=src.tensor,
                    offset=src.offset,
                    ap=[[0, BINS], src.ap[0], src.ap[1]],
                )
                dt_g = work.tile([128, K], F32, tag="dt")
                nc.sync.dma_start(out=dt_g, in_=bsrc)
                t2_g = work.tile([128, K], F32, tag="t2")
                nc.scalar.activation(
                    out=t2_g, in_=dt_g, func=AF.Square,
                    bias=geo_bias, scale=1.0 / WIDTH,
                )
                dst = rbf_fast[:, g * K : (g + 1) * K]
                nc.scalar.activation(out=dst, in_=t2_g, func=AF.Exp, scale=-0.5)
                if apply_mask_c:
                    nc.vector.tensor_scalar_mul(
                        out=dst, in0=dst, scalar1=mcm_t[:, g : g + 1]
                    )
                    nc.vector.tensor_mul(out=dst, in0=dst, in1=mca_t)

            # m-interleaved grouping: partition-slot (g, mi) holds logical
            # m = mi*12 + g, so (g, a) merges into one contiguous dim and the
            # permute DMAs fit the 3-dim AP limit.
            # geo output: geo_o[k, m, a] <- rbf_fast[(k,mi), (g,a)]
            geo_full = geo_o[:]
            geo_dst = bass.AP(
                tensor=geo_full.tensor,
                offset=geo_full.offset,
                ap=[[MB * K, BINS], [G * K, 8], [1, G * K]],
            )
            nc.sync.dma_start(out=geo_dst, in_=rbf_fast[:])
            # permute into rbfz[k, m*384 + a]
            rbfz = const.tile([BINS, MB * K], F16, tag="rbfz")
            for k in range(BINS):
                dst = rbfz[k : k + 1, :]
                dst = bass.AP(
                    tensor=dst.tensor,
                    offset=dst.offset,
                    ap=[dst.ap[0], [G * K, 8], [1, G * K]],
                )
                nc.sync.dma_start(out=dst, in_=rbf_fast[8 * k : 8 * k + 8, :])

            # ---- main loop ----
            for g in range(G):
                zt = []
                for nb in range(NB):
                    t = zpool.tile([128, 8 * CZ], F16, tag=f"z_{nb}")
                    nc.sync.dma_start(
                        out=t,
                        in_=z_e[nb * 128 : (nb + 1) * 128,
                                g * 8 * CZ : (g + 1) * 8 * CZ],
                    )
                    zt.append(t)
                for mi in range(8):
                    m = 12 * mi + g
                    zsem_ps = ps_a.tile([CZ, K], F32, tag="zsem")
                    for nb in range(NB):
                        nc.tensor.matmul(
                            zsem_ps,
                            zt[nb][:, mi * CZ : (mi + 1) * CZ],
                            a3_t[nb],
                            start=(nb == 0),
                            stop=(nb == NB - 1),
                        )
                    zsem16 = work.tile([CZ, K], F16, tag="zsem16")
                    nc.vector.tensor_copy(out=zsem16, in_=zsem_ps)
                    nc.sync.dma_start(out=zsem_o[m], in_=zsem16)

                    h_ps = ps_a.tile([COUT, K], F32, tag="h")
                    nc.tensor.matmul(h_ps, w1a_t, zsem16, start=True, stop=False)
                    nc.tensor.matmul(
                        h_ps, w1b_t, rbfz[:, m * K : (m + 1) * K],
                        start=False, stop=True,
                    )
                    h16 = work.tile([COUT, K], F16, tag="h16")
                    nc.scalar.activation(out=h16, in_=h_ps, func=AF.Identity, bias=b1c)

                    hc_ps = ps_a.tile([COUT, K], F32, tag="hc")
                    nc.tensor.matmul(hc_ps, cmat_t, h16, start=True, stop=True)
                    hcsq16 = work.tile([COUT, K], F16, tag="hcsq")
                    nc.scalar.activation(out=hcsq16, in_=hc_ps, func=AF.Square)

                    var_ps = ps_b.tile([COUT, K], F32, tag="var")
                    nc.tensor.matmul(var_ps, jmat_t, hcsq16, start=True, stop=True)
                    s_sb = work.tile([COUT, K], F32, tag="s_sb")
                    nc.scalar.activation(out=s_sb, in_=var_ps, func=AF.Sqrt, bias=epsc)

                    r_sb = work.tile([COUT, K], F32, tag="r_sb")
                    nc.vector.reciprocal(out=r_sb, in_=s_sb)
                    t_sb = work.tile([COUT, K], F32, tag="t_sb")
                    nc.vector.tensor_tensor(
                        out=t_sb, in0=hc_ps, in1=r_sb, op=mybir.AluOpType.mult
                    )
                    s16 = work.tile([COUT, K], F16, tag="s16")
                    nc.scalar.activation(
                        out=s16, in_=t_sb, func=AF.Silu, bias=bc, scale=gc
                    )

                    o_ps = ps_b.tile([COUT, K], F32, tag="out")
                    nc.tensor.matmul(o_ps, w2_t, s16, start=True, stop=True)
                    o16 = work.tile([COUT, K], F16, tag="o16")
                    nc.vector.tensor_scalar_add(out=o16, in0=o_ps, scalar1=b2c)
                    nc.sync.dma_start(out=zc_o[m], in_=o16)

    sys.path.insert(0, "/root/problem")
    try:
        from wait_split import split_multi_waits
    except ImportError:
        split_multi_waits = _split_multi_waits_inline
    split_multi_waits(nc)
    return nc


def _split_multi_waits_inline(nc, max_waits: int = 1):
    for fn in nc.m.functions:
        for blk in fn.blocks:
            insts = list(blk.instructions)
            out = []
            changed = False
            for inst in insts:
                si = inst.sync_info
                waits = list(si.on_wait) if (si is not None and si.on_wait) else []
                limit = 2 if isinstance(inst, mybir.InstEventSemaphore) else max_waits
                if len(waits) > limit:
                    changed = True
                    for w in waits[: len(waits) - limit]:
                        nop = mybir.InstNoOp(
                            name=nc.get_next_instruction_name(), ins=[], outs=[]
                        )
                        nop.engine = inst.engine
                        nop.sync_info = mybir.SyncInfo(on_wait=[w], on_update=[])
                        out.append(nop)
                    si.on_wait = waits[len(waits) - limit :]
                out.append(inst)
            if changed:
                try:
                    blk.instructions[:] = out
                except TypeError:
                    blk.instructions = out


def kernel(A, Z_fine, mu_c, mask_f, mask_c, W1, b1, ln_g, ln_b, W2, b2):
    global LAST_RUN_SECONDS
    A = np.asarray(A, np.float32)
    Z_fine = np.asarray(Z_fine, np.float32)
    mu_c = np.asarray(mu_c, np.float32)
    mask_f = np.asarray(mask_f, np.float32)
    mask_c = np.asarray(mask_c, np.float32)
    W1 = np.asarray(W1, np.float32)
    b1v = np.asarray(b1, np.float32)
    ln_gv = np.asarray(ln_g, np.float32)
    ln_bv = np.asarray(ln_b, np.float32)
    W2m = np.asarray(W2, np.float32)
    b2v = np.asarray(b2, np.float32)

    # ---- host prep ----
    Am = A * mask_f[:, :, None]
    s = Am.sum(-1)                       # [B, m]
    col = Am.sum(1)                      # [B, a]
    q = (mask_f[:, None, :].astype(np.float64) * s[:, None, :]) / np.maximum(
        col[:, :, None].astype(np.float64) * s[:, None, :], EPS
    )
    with np.errstate(divide="ignore", invalid="ignore"):
        inv_col = np.where(col != 0.0, 1.0 / col.astype(np.float64), 0.0)
    fact = mask_f[:, None, :] * inv_col[:, :, None]
    if not np.allclose(q, fact, rtol=1e-5, atol=1e-12):
        raise NotImplementedError("EPS-clamped non-separable Z_den")

    A3 = A * (mask_f**2)[:, :, None] * inv_col[:, None, :].astype(np.float32)
    A3 = A3.astype(np.float16)           # [B, n, a]

    delta = mu_c[:, :, None, :] - mu_c[:, None, :, :]
    dfull = np.sqrt((delta.astype(np.float64) ** 2).sum(-1) + EPS).astype(np.float32)

    centers = np.linspace(0.0, DMAX, BINS, dtype=np.float64)
    cvec = np.zeros((128, 8), np.float32)
    cvec[:, 0] = np.repeat(-centers / WIDTH, 8)
    cvec[:, 1] = b1v
    cvec[:, 2] = b2v
    cvec[:, 3] = ln_gv
    cvec[:, 4] = ln_bv
    cvec[:, 5] = LN_EPS

    w1a = W1[:CZ, :].astype(np.float16)
    w1b = W1[CZ:, :].astype(np.float16)
    w2 = W2m.astype(np.float16)
    cmat = (np.eye(128, dtype=np.float64) - 1.0 / 128).astype(np.float16)
    jmat = np.full((128, 128), 1.0 / 128, np.float16)

    apply_mask_c = not np.all(mask_c == 1.0)
    nc = _build_program(apply_mask_c)

    # group-slot p = g*8 + mi holds logical m = mi*12 + g
    m_order = np.array([mi * 12 + g for g in range(G) for mi in range(8)])
    in_maps = []
    for core in range(8):
        b, mb = divmod(core, 4)
        ms, me = mb * MB, (mb + 1) * MB
        zs = Z_fine[b, :, ms:me, :] * mask_f[b, ms:me][None, :, None]
        zs = zs[:, m_order, :]
        im = {
            "a3": np.ascontiguousarray(
                A3[b].reshape(NB, 128, K)
            ),
            "z": np.ascontiguousarray(zs.reshape(N, MB * CZ).astype(np.float16)),
            "dmat": np.ascontiguousarray(dfull[b, ms:me, :][m_order]),
            "cvec": cvec,
            "w1a": w1a, "w1b": w1b, "w2": w2, "cmat": cmat, "jmat": jmat,
        }
        if apply_mask_c:
            mcm = np.zeros((128, G), np.float32)
            for p in range(128):
                for g in range(G):
                    mcm[p, g] = mask_c[b, ms + (p % 8) * 12 + g]
            im["mcm"] = mcm
            im["mca"] = np.broadcast_to(mask_c[b], (128, K)).copy()
        in_maps.append(im)

    t0 = time.perf_counter()
    res = run_bass_kernel_spmd(nc, in_maps, list(range(8)))
    LAST_RUN_SECONDS = time.perf_counter() - t0

    Z_sem = np.empty((B, K, N, CZ), np.float32)
    Z_c = np.empty((B, K, N, COUT), np.float32)
    Z_geo = np.empty((B, K, K, BINS), np.float32)
    for core in range(8):
        b, mb = divmod(core, 4)
        ms, me = mb * MB, (mb + 1) * MB
        r = res.results[core]
        Z_sem[b, :, ms:me, :] = r["zsem_o"].astype(np.float32).transpose(2, 0, 1)
        Z_c[b, :, ms:me, :] = r["zc_o"].astype(np.float32).transpose(2, 0, 1)
        Z_geo[b, :, ms:me, :] = r["geo_o"].astype(np.float32).transpose(2, 1, 0)
    if apply_mask_c:
        mc2 = mask_c[:, :, None] * mask_c[:, None, :]
        Z_c *= mc2[..., None]
    return (Z_c, Z_sem, Z_geo)
